# revision 18
# baseline (speedup 1.0000x reference)
"""Trainium2 Bass kernel for nn_MetaMultiParallelMLP.

Per-group MLP: G=16 groups, 8 hidden layers of width 64 (ReLU), skip-concat
of the 3-channel input after layer 4, then a 64->4 output layer.

Strategy:
  - Data parallel over the points axis N=65536 across 8 cores (8192 pts/core).
  - On each core, pack 2 groups into the 128 SBUF partitions (8 "pairs").
    Activations live channel-major [C, n]; weights are block-diagonal
    [128, 128] (two 64x64 blocks), so one matmul advances both groups.
  - The skip concat is never materialized: layer 5 is two matmuls
    accumulating in PSUM (K=128 h-part + K=6 x-part).
  - Matmuls run in bf16 (full PE rate; fp32 PSUM accumulate, fp32 biases).
  - Work is issued layer-round-robin over the 8 independent pair chains so
    PE / DVE / ACT all stay busy; PSUM units are [128,1024] (2 banks x 4
    slots). Bias+ReLU is fused into the PSUM->SBUF copy, statically split
    between the scalar and vector engines ~9:7 (ACT is faster/element).
  - Output layer packs 2 pairs per PSUM tile as M=64 matmuls at partition
    offsets 0/64 (zero-padded weights), so bias-add+copy batches 2 pairs.

Host side packs/unpacks layouts with numpy; device time is what counts.
"""

import ml_dtypes
import numpy as np

BF16_NP = ml_dtypes.bfloat16

import concourse.bass as bass
import concourse.mybir as mybir
import concourse.tile as tile
from concourse import bacc
from concourse.bass_utils import run_bass_kernel_spmd

G, D, W, IN_CH, OUT_CH = 16, 8, 64, 3, 4
B, N = 1, 65536
NCORES = 8
NC_PTS = N // NCORES  # 8192 points per core
NPAIR = G // 2  # 8 group-pairs, 2x64 channels = 128 partitions
PT = 512  # points per matmul (fp32 PSUM bank limit)
UW = 2048  # unit width: 4 matmuls -> one PSUM unit [128, 2048] (4 banks)
TGW = 2048  # t-group width per pair (2 units)
NTG = NC_PTS // TGW  # 4 t-groups
NMID = 7  # full-K layers: L1..L4, L5h, L6, L7

F32 = mybir.dt.float32
BF16 = mybir.dt.bfloat16


def _act_engine(k):
    # strict alternation keeps both engines fed; +4/64 bias toward ACT
    # (ACT is ~12% faster per element than DVE for PSUM-source ops)
    return "act" if (k % 2 == 0 or k % 16 == 1) else "dve"


def _build_nc():
    nc = bacc.Bacc(None, target_bir_lowering=False)

    xin = nc.dram_tensor("xin", [NPAIR, 6, NC_PTS], BF16, kind="ExternalInput")
    wf_d = nc.dram_tensor("w_first", [NPAIR, 6, 128], BF16, kind="ExternalInput")
    wm_d = nc.dram_tensor("w_mid", [NMID, NPAIR, 128, 128], BF16, kind="ExternalInput")
    w5x_d = nc.dram_tensor("w5x", [NPAIR, 6, 128], BF16, kind="ExternalInput")
    wo_d = nc.dram_tensor("w_out", [NPAIR, 128, 64], BF16, kind="ExternalInput")
    bh_d = nc.dram_tensor("bias_hid", [128, 8 * NPAIR], F32, kind="ExternalInput")
    bo_d = nc.dram_tensor("bias_out", [128, 4], F32, kind="ExternalInput")

    hout = nc.dram_tensor("h_out", [NPAIR, 128, NC_PTS], BF16, kind="ExternalOutput")
    yout = nc.dram_tensor("y_out", [NPAIR, 8, NC_PTS], F32, kind="ExternalOutput")

    relu = mybir.ActivationFunctionType.Relu
    alu_add = mybir.AluOpType.add
    alu_max = mybir.AluOpType.max

    with tile.TileContext(nc) as tc:
        with (
            tc.tile_pool(name="weights", bufs=1) as wpool,
            tc.tile_pool(name="x", bufs=18) as xpool,
            tc.tile_pool(name="h", bufs=16) as hpool,
            tc.tile_pool(name="y", bufs=6) as ypool,
            tc.tile_pool(name="ps", bufs=2, space=bass.MemorySpace.PSUM) as pspool,
        ):
            # ---- stage all weights/biases into SBUF (one-time) ----
            # order matters at startup: L0 weights + biases first so compute
            # can begin while the bulk mid-layer weights stream in
            wf_sb = wpool.tile([6, NPAIR * 128], BF16, tag="wf")
            for p in range(NPAIR):
                nc.sync.dma_start(wf_sb[:, p * 128 : (p + 1) * 128], wf_d[p])
            bh_sb = wpool.tile([128, 8 * NPAIR], F32, tag="bh")
            nc.sync.dma_start(bh_sb[:], bh_d[:])
            bo_sb = wpool.tile([128, 4], F32, tag="bo")
            nc.sync.dma_start(bo_sb[:], bo_d[:])
            wm_sb = wpool.tile([128, NMID * NPAIR * 128], BF16, tag="wm")
            for l in range(NMID):
                for p in range(NPAIR):
                    c0 = (l * NPAIR + p) * 128
                    nc.sync.dma_start(wm_sb[:, c0 : c0 + 128], wm_d[l, p])
            w5x_sb = wpool.tile([6, NPAIR * 128], BF16, tag="w5x")
            for p in range(NPAIR):
                nc.sync.dma_start(w5x_sb[:, p * 128 : (p + 1) * 128], w5x_d[p])
            wo_sb = wpool.tile([128, NPAIR * 64], BF16, tag="wo")
            for p in range(NPAIR):
                nc.sync.dma_start(wo_sb[:, p * 64 : (p + 1) * 64], wo_d[p])

            def wmid(l_mid, p):
                c0 = (l_mid * NPAIR + p) * 128
                return wm_sb[:, c0 : c0 + 128]

            def act_relu_bias(k, dst, src, bias_ap):
                # dst = relu(src + bias)
                if _act_engine(k) == "dve":
                    nc.vector.tensor_scalar(dst, src, bias_ap, 0.0, alu_add, alu_max)
                else:
                    nc.scalar.activation(dst, src, relu, bias=bias_ap)

            # ---- main loop: 4 t-groups of 2048 points x 8 pairs ----
            for tg in range(NTG):
                n0 = tg * TGW
                xts = []
                for p in range(NPAIR):
                    xt = xpool.tile([6, TGW], BF16, tag="xt")
                    nc.sync.dma_start(xt[:], xin[p, :, n0 : n0 + TGW])
                    xts.append(xt)

                hs = [None] * NPAIR
                kcnt = 0
                for l in range(D):
                    bcol = l * NPAIR
                    for p in range(NPAIR):
                        hn = hpool.tile([128, TGW], BF16, tag="h")
                        bias_ap = bh_sb[:, bcol + p : bcol + p + 1]
                        for u in range(TGW // UW):
                            ps = pspool.tile([128, UW], F32, tag="ps")
                            for s in range(UW // PT):
                                lo = u * UW + s * PT
                                dst = ps[:, s * PT : (s + 1) * PT]
                                if l == 0:
                                    nc.tensor.matmul(
                                        dst,
                                        wf_sb[:, p * 128 : (p + 1) * 128],
                                        xts[p][:, lo : lo + PT],
                                        start=True, stop=True,
                                    )
                                elif l == 5:
                                    nc.tensor.matmul(
                                        dst, wmid(4, p), hs[p][:, lo : lo + PT],
                                        start=True, stop=False,
                                    )
                                    nc.tensor.matmul(
                                        dst,
                                        w5x_sb[:, p * 128 : (p + 1) * 128],
                                        xts[p][:, lo : lo + PT],
                                        start=False, stop=True,
                                    )
                                else:
                                    lm = l - 1  # mids: [w1..w4, w5h, w6, w7]
                                    nc.tensor.matmul(
                                        dst, wmid(lm, p), hs[p][:, lo : lo + PT],
                                        start=True, stop=True,
                                    )
                            act_relu_bias(
                                kcnt, hn[:, u * UW : (u + 1) * UW], ps[:], bias_ap
                            )
                            kcnt += 1
                        hs[p] = hn

                        if l != D - 1:
                            continue
                        # layer 7 done for this pair: stream out h7 and, per
                        # duo, the output layer — interleaved so the PE never
                        # drains at the t-group tail (keeps the HAM clock warm)
                        nc.sync.dma_start(hout[p, :, n0 : n0 + TGW], hs[p][:])
                        if p % 2 == 0:
                            continue
                        dd = p // 2
                        yt = ypool.tile([128, TGW], F32, tag="yt")
                        for u in range(TGW // UW):
                            pso = pspool.tile([128, UW], F32, tag="ps")
                            for e in range(2):
                                pp = 2 * dd + e
                                for s in range(UW // PT):
                                    lo = u * UW + s * PT
                                    nc.tensor.matmul(
                                        pso[64 * e : 64 * e + 64, s * PT : (s + 1) * PT],
                                        wo_sb[:, pp * 64 : (pp + 1) * 64],
                                        hs[pp][:, lo : lo + PT],
                                        start=True, stop=True,
                                    )
                            # bias-add + copy (junk rows beyond 8/72 harmless)
                            if (dd + u) % 2 == 0:
                                nc.vector.tensor_scalar_add(
                                    yt[:, u * UW : (u + 1) * UW], pso[:],
                                    bo_sb[:, dd : dd + 1],
                                )
                            else:
                                nc.scalar.add(
                                    yt[:, u * UW : (u + 1) * UW], pso[:],
                                    bo_sb[:, dd : dd + 1],
                                )
                        nc.sync.dma_start(yout[2 * dd, :, n0 : n0 + TGW], yt[0:8, :])
                        nc.sync.dma_start(
                            yout[2 * dd + 1, :, n0 : n0 + TGW], yt[64:72, :]
                        )

    nc.compile()
    return nc


_CACHE = {}


def _get_nc():
    if "nc" not in _CACHE:
        _CACHE["nc"] = _build_nc()
    return _CACHE["nc"]


def pack_params(inputs):
    """Block-diagonal pair packing of all weights/biases (host, one-time)."""
    ws = [np.asarray(inputs[f"w{i}"], np.float32) for i in range(D)]
    bs = [np.asarray(inputs[f"b{i}"], np.float32) for i in range(D)]
    w_out = np.asarray(inputs["w_out"], np.float32)
    b_out = np.asarray(inputs["b_out"], np.float32)

    wf = np.zeros((NPAIR, 6, 128), np.float32)
    w5x = np.zeros((NPAIR, 6, 128), np.float32)
    wo = np.zeros((NPAIR, 128, 64), np.float32)
    for p in range(NPAIR):
        wf[p, 0:3, 0:64] = ws[0][2 * p].T
        wf[p, 3:6, 64:128] = ws[0][2 * p + 1].T
        w5x[p, 0:3, 0:64] = ws[5][2 * p, :, 0:3].T
        w5x[p, 3:6, 64:128] = ws[5][2 * p + 1, :, 0:3].T
        wo[p, 0:64, 0:4] = w_out[2 * p].T
        wo[p, 64:128, 4:8] = w_out[2 * p + 1].T

    mids = [ws[1], ws[2], ws[3], ws[4], ws[5][:, :, 3:67], ws[6], ws[7]]
    wm = np.zeros((NMID, NPAIR, 128, 128), np.float32)
    for l, wl in enumerate(mids):
        for p in range(NPAIR):
            wm[l, p, 0:64, 0:64] = wl[2 * p].T
            wm[l, p, 64:128, 64:128] = wl[2 * p + 1].T

    bh = np.zeros((128, 8 * NPAIR), np.float32)
    for l in range(D):
        for p in range(NPAIR):
            bh[0:64, l * NPAIR + p] = bs[l][2 * p]
            bh[64:128, l * NPAIR + p] = bs[l][2 * p + 1]
    bo = np.zeros((128, 4), np.float32)
    for dd in range(4):
        bo[0:4, dd] = b_out[4 * dd]
        bo[4:8, dd] = b_out[4 * dd + 1]
        bo[64:68, dd] = b_out[4 * dd + 2]
        bo[68:72, dd] = b_out[4 * dd + 3]

    return {
        "w_first": wf.astype(BF16_NP),
        "w_mid": wm.astype(BF16_NP),
        "w5x": w5x.astype(BF16_NP),
        "w_out": wo.astype(BF16_NP),
        "bias_hid": bh,
        "bias_out": bo,
    }


def shard_x(x):
    """x (1,G,N,3) -> per-core list of (NPAIR, 6, NC_PTS) channel-major."""
    xr = np.asarray(x, np.float32)[0].reshape(G, NCORES, NC_PTS, IN_CH)
    out = []
    for c in range(NCORES):
        xc = xr[:, c]  # (G, NC_PTS, 3)
        xp = (
            xc.reshape(NPAIR, 2, NC_PTS, IN_CH)
            .transpose(0, 1, 3, 2)
            .reshape(NPAIR, 6, NC_PTS)
        )
        out.append(np.ascontiguousarray(xp).astype(BF16_NP))
    return out


def unshard(results):
    """Per-core {h_out,y_out} -> full (1,G,N,4) and (1,G,N,64)."""
    h_full = np.empty((B, G, N, W), np.float32)
    y_full = np.empty((B, G, N, OUT_CH), np.float32)
    for c, r in enumerate(results):
        hc = (
            np.asarray(r["h_out"]).astype(np.float32)
            .reshape(NPAIR, 2, W, NC_PTS)
            .transpose(0, 1, 3, 2)
            .reshape(G, NC_PTS, W)
        )
        yc = (
            np.asarray(r["y_out"], np.float32)
            .reshape(NPAIR, 2, OUT_CH, NC_PTS)
            .transpose(0, 1, 3, 2)
            .reshape(G, NC_PTS, OUT_CH)
        )
        sl = slice(c * NC_PTS, (c + 1) * NC_PTS)
        h_full[0, :, sl, :] = hc
        y_full[0, :, sl, :] = yc
    return y_full, h_full


def run_on_hw(inputs, **spmd_kwargs):
    params = pack_params(inputs)
    xs = shard_x(inputs["x"])
    in_maps = [{"xin": xs[c], **params} for c in range(NCORES)]
    res = run_bass_kernel_spmd(_get_nc(), in_maps, list(range(NCORES)), **spmd_kwargs)
    return res


def kernel(**inputs):
    res = run_on_hw(inputs)
    return unshard(res.results)


# revision 19
# speedup vs baseline: 1.4625x; 1.4625x over previous
"""Trainium2 Bass kernel for nn_MetaMultiParallelMLP.

Per-group MLP: G=16 groups, 8 hidden layers of width 64 (ReLU), skip-concat
of the 3-channel input after layer 4, then a 64->4 output layer.

Strategy:
  - Data parallel over the points axis N=65536 across 8 cores (8192 pts/core).
  - On each core, pack 2 groups into the 128 SBUF partitions (8 "pairs").
    Activations live channel-major [C, n]; weights are block-diagonal
    [128, 128] (two 64x64 blocks), so one matmul advances both groups.
  - The skip concat is never materialized: layer 5 is two matmuls
    accumulating in PSUM (K=128 h-part + K=6 x-part).
  - Matmuls run in bf16 (full PE rate; fp32 PSUM accumulate, fp32 biases).
  - Work is issued layer-round-robin over the 8 independent pair chains so
    PE / DVE / ACT all stay busy; PSUM units are [128,1024] (2 banks x 4
    slots). Bias+ReLU is fused into the PSUM->SBUF copy, statically split
    between the scalar and vector engines ~9:7 (ACT is faster/element).
  - Output layer packs 2 pairs per PSUM tile as M=64 matmuls at partition
    offsets 0/64 (zero-padded weights), so bias-add+copy batches 2 pairs.

Host side packs/unpacks layouts with numpy; device time is what counts.
"""

import ml_dtypes
import numpy as np

BF16_NP = ml_dtypes.bfloat16

import concourse.bass as bass
import concourse.mybir as mybir
import concourse.tile as tile
from concourse import bacc
from concourse.bass_utils import run_bass_kernel_spmd

G, D, W, IN_CH, OUT_CH = 16, 8, 64, 3, 4
B, N = 1, 65536
NCORES = 8
NC_PTS = N // NCORES  # 8192 points per core
NPAIR = G // 2  # 8 group-pairs, 2x64 channels = 128 partitions
PT = 512  # points per matmul (fp32 PSUM bank limit)
UW = 1024  # unit width: 2 matmuls -> one PSUM unit [128, 1024] (2 banks)
TGW = 2048  # t-group width per pair (2 units)
NTG = NC_PTS // TGW  # 4 t-groups
NMID = 7  # full-K layers: L1..L4, L5h, L6, L7

F32 = mybir.dt.float32
BF16 = mybir.dt.bfloat16


def _act_engine(k):
    # strict alternation keeps both engines fed; +4/64 bias toward ACT
    # (ACT is ~12% faster per element than DVE for PSUM-source ops)
    return "act" if (k % 2 == 0 or k % 32 == 1) else "dve"


def _build_nc():
    nc = bacc.Bacc(None, target_bir_lowering=False)

    xin = nc.dram_tensor("xin", [NPAIR, 6, NC_PTS], BF16, kind="ExternalInput")
    wf_d = nc.dram_tensor("w_first", [NPAIR, 6, 128], BF16, kind="ExternalInput")
    wm_d = nc.dram_tensor("w_mid", [NMID, NPAIR, 128, 128], BF16, kind="ExternalInput")
    w5x_d = nc.dram_tensor("w5x", [NPAIR, 6, 128], BF16, kind="ExternalInput")
    wo_d = nc.dram_tensor("w_out", [NPAIR, 128, 64], BF16, kind="ExternalInput")
    bh_d = nc.dram_tensor("bias_hid", [128, 8 * NPAIR], F32, kind="ExternalInput")
    bo_d = nc.dram_tensor("bias_out", [128, 4], F32, kind="ExternalInput")

    hout = nc.dram_tensor("h_out", [NPAIR, 128, NC_PTS], BF16, kind="ExternalOutput")
    yout = nc.dram_tensor("y_out", [NPAIR, 8, NC_PTS], F32, kind="ExternalOutput")

    relu = mybir.ActivationFunctionType.Relu
    alu_add = mybir.AluOpType.add
    alu_max = mybir.AluOpType.max

    with tile.TileContext(nc) as tc:
        with (
            tc.tile_pool(name="weights", bufs=1) as wpool,
            tc.tile_pool(name="x", bufs=18) as xpool,
            tc.tile_pool(name="h", bufs=16) as hpool,
            tc.tile_pool(name="y", bufs=6) as ypool,
            tc.tile_pool(name="ps", bufs=4, space=bass.MemorySpace.PSUM) as pspool,
        ):
            # ---- stage all weights/biases into SBUF (one-time) ----
            # order matters at startup: L0 weights + biases first so compute
            # can begin while the bulk mid-layer weights stream in
            wf_sb = wpool.tile([6, NPAIR * 128], BF16, tag="wf")
            for p in range(NPAIR):
                nc.sync.dma_start(wf_sb[:, p * 128 : (p + 1) * 128], wf_d[p])
            bh_sb = wpool.tile([128, 8 * NPAIR], F32, tag="bh")
            nc.sync.dma_start(bh_sb[:], bh_d[:])
            bo_sb = wpool.tile([128, 4], F32, tag="bo")
            nc.sync.dma_start(bo_sb[:], bo_d[:])
            wm_sb = wpool.tile([128, NMID * NPAIR * 128], BF16, tag="wm")
            for l in range(NMID):
                for p in range(NPAIR):
                    c0 = (l * NPAIR + p) * 128
                    nc.sync.dma_start(wm_sb[:, c0 : c0 + 128], wm_d[l, p])
            w5x_sb = wpool.tile([6, NPAIR * 128], BF16, tag="w5x")
            for p in range(NPAIR):
                nc.sync.dma_start(w5x_sb[:, p * 128 : (p + 1) * 128], w5x_d[p])
            wo_sb = wpool.tile([128, NPAIR * 64], BF16, tag="wo")
            for p in range(NPAIR):
                nc.sync.dma_start(wo_sb[:, p * 64 : (p + 1) * 64], wo_d[p])

            def wmid(l_mid, p):
                c0 = (l_mid * NPAIR + p) * 128
                return wm_sb[:, c0 : c0 + 128]

            def act_relu_bias(k, dst, src, bias_ap):
                # dst = relu(src + bias)
                if _act_engine(k) == "dve":
                    nc.vector.tensor_scalar(dst, src, bias_ap, 0.0, alu_add, alu_max)
                else:
                    nc.scalar.activation(dst, src, relu, bias=bias_ap)

            # ---- main loop: 4 t-groups of 2048 points x 8 pairs ----
            for tg in range(NTG):
                n0 = tg * TGW
                xts = []
                for p in range(NPAIR):
                    xt = xpool.tile([6, TGW], BF16, tag="xt")
                    nc.sync.dma_start(xt[:], xin[p, :, n0 : n0 + TGW])
                    xts.append(xt)

                hs = [None] * NPAIR
                kcnt = 0
                for l in range(D):
                    bcol = l * NPAIR
                    for p in range(NPAIR):
                        hn = hpool.tile([128, TGW], BF16, tag="h")
                        bias_ap = bh_sb[:, bcol + p : bcol + p + 1]
                        for u in range(TGW // UW):
                            ps = pspool.tile([128, UW], F32, tag="ps")
                            if l == 5:
                                for s in range(UW // PT):
                                    lo = u * UW + s * PT
                                    nc.tensor.matmul(
                                        ps[:, s * PT : (s + 1) * PT],
                                        wmid(4, p), hs[p][:, lo : lo + PT],
                                        start=True, stop=False,
                                    )
                                for s in range(UW // PT):
                                    lo = u * UW + s * PT
                                    nc.tensor.matmul(
                                        ps[:, s * PT : (s + 1) * PT],
                                        w5x_sb[:, p * 128 : (p + 1) * 128],
                                        xts[p][:, lo : lo + PT],
                                        start=False, stop=True,
                                    )
                            else:
                                for s in range(UW // PT):
                                    lo = u * UW + s * PT
                                    dst = ps[:, s * PT : (s + 1) * PT]
                                    if l == 0:
                                        nc.tensor.matmul(
                                            dst,
                                            wf_sb[:, p * 128 : (p + 1) * 128],
                                            xts[p][:, lo : lo + PT],
                                            start=True, stop=True,
                                        )
                                    else:
                                        lm = l - 1  # mids: [w1..w4, w5h, w6, w7]
                                        nc.tensor.matmul(
                                            dst, wmid(lm, p), hs[p][:, lo : lo + PT],
                                            start=True, stop=True,
                                        )
                            act_relu_bias(
                                kcnt, hn[:, u * UW : (u + 1) * UW], ps[:], bias_ap
                            )
                            kcnt += 1
                        hs[p] = hn

                        if l != D - 1:
                            continue
                        # layer 7 done for this pair: stream out h7 and, per
                        # duo, the output layer — interleaved so the PE never
                        # drains at the t-group tail (keeps the HAM clock warm)
                        nc.sync.dma_start(hout[p, :, n0 : n0 + TGW], hs[p][:])
                        if p % 2 == 0:
                            continue
                        dd = p // 2
                        yt = ypool.tile([128, TGW], F32, tag="yt")
                        for u in range(TGW // UW):
                            pso = pspool.tile([128, UW], F32, tag="ps")
                            for e in range(2):
                                pp = 2 * dd + e
                                for s in range(UW // PT):
                                    lo = u * UW + s * PT
                                    nc.tensor.matmul(
                                        pso[64 * e : 64 * e + 64, s * PT : (s + 1) * PT],
                                        wo_sb[:, pp * 64 : (pp + 1) * 64],
                                        hs[pp][:, lo : lo + PT],
                                        start=True, stop=True,
                                    )
                            # bias-add + copy (junk rows beyond 8/72 harmless)
                            if (dd + u) % 2 == 0:
                                nc.vector.tensor_scalar_add(
                                    yt[:, u * UW : (u + 1) * UW], pso[:],
                                    bo_sb[:, dd : dd + 1],
                                )
                            else:
                                nc.scalar.add(
                                    yt[:, u * UW : (u + 1) * UW], pso[:],
                                    bo_sb[:, dd : dd + 1],
                                )
                        nc.sync.dma_start(yout[2 * dd, :, n0 : n0 + TGW], yt[0:8, :])
                        nc.sync.dma_start(
                            yout[2 * dd + 1, :, n0 : n0 + TGW], yt[64:72, :]
                        )

    nc.compile()
    return nc


_CACHE = {}


def _get_nc():
    if "nc" not in _CACHE:
        _CACHE["nc"] = _build_nc()
    return _CACHE["nc"]


def pack_params(inputs):
    """Block-diagonal pair packing of all weights/biases (host, one-time)."""
    ws = [np.asarray(inputs[f"w{i}"], np.float32) for i in range(D)]
    bs = [np.asarray(inputs[f"b{i}"], np.float32) for i in range(D)]
    w_out = np.asarray(inputs["w_out"], np.float32)
    b_out = np.asarray(inputs["b_out"], np.float32)

    wf = np.zeros((NPAIR, 6, 128), np.float32)
    w5x = np.zeros((NPAIR, 6, 128), np.float32)
    wo = np.zeros((NPAIR, 128, 64), np.float32)
    for p in range(NPAIR):
        wf[p, 0:3, 0:64] = ws[0][2 * p].T
        wf[p, 3:6, 64:128] = ws[0][2 * p + 1].T
        w5x[p, 0:3, 0:64] = ws[5][2 * p, :, 0:3].T
        w5x[p, 3:6, 64:128] = ws[5][2 * p + 1, :, 0:3].T
        wo[p, 0:64, 0:4] = w_out[2 * p].T
        wo[p, 64:128, 4:8] = w_out[2 * p + 1].T

    mids = [ws[1], ws[2], ws[3], ws[4], ws[5][:, :, 3:67], ws[6], ws[7]]
    wm = np.zeros((NMID, NPAIR, 128, 128), np.float32)
    for l, wl in enumerate(mids):
        for p in range(NPAIR):
            wm[l, p, 0:64, 0:64] = wl[2 * p].T
            wm[l, p, 64:128, 64:128] = wl[2 * p + 1].T

    bh = np.zeros((128, 8 * NPAIR), np.float32)
    for l in range(D):
        for p in range(NPAIR):
            bh[0:64, l * NPAIR + p] = bs[l][2 * p]
            bh[64:128, l * NPAIR + p] = bs[l][2 * p + 1]
    bo = np.zeros((128, 4), np.float32)
    for dd in range(4):
        bo[0:4, dd] = b_out[4 * dd]
        bo[4:8, dd] = b_out[4 * dd + 1]
        bo[64:68, dd] = b_out[4 * dd + 2]
        bo[68:72, dd] = b_out[4 * dd + 3]

    return {
        "w_first": wf.astype(BF16_NP),
        "w_mid": wm.astype(BF16_NP),
        "w5x": w5x.astype(BF16_NP),
        "w_out": wo.astype(BF16_NP),
        "bias_hid": bh,
        "bias_out": bo,
    }


def shard_x(x):
    """x (1,G,N,3) -> per-core list of (NPAIR, 6, NC_PTS) channel-major."""
    xr = np.asarray(x, np.float32)[0].reshape(G, NCORES, NC_PTS, IN_CH)
    out = []
    for c in range(NCORES):
        xc = xr[:, c]  # (G, NC_PTS, 3)
        xp = (
            xc.reshape(NPAIR, 2, NC_PTS, IN_CH)
            .transpose(0, 1, 3, 2)
            .reshape(NPAIR, 6, NC_PTS)
        )
        out.append(np.ascontiguousarray(xp).astype(BF16_NP))
    return out


def unshard(results):
    """Per-core {h_out,y_out} -> full (1,G,N,4) and (1,G,N,64)."""
    h_full = np.empty((B, G, N, W), np.float32)
    y_full = np.empty((B, G, N, OUT_CH), np.float32)
    for c, r in enumerate(results):
        hc = (
            np.asarray(r["h_out"]).astype(np.float32)
            .reshape(NPAIR, 2, W, NC_PTS)
            .transpose(0, 1, 3, 2)
            .reshape(G, NC_PTS, W)
        )
        yc = (
            np.asarray(r["y_out"], np.float32)
            .reshape(NPAIR, 2, OUT_CH, NC_PTS)
            .transpose(0, 1, 3, 2)
            .reshape(G, NC_PTS, OUT_CH)
        )
        sl = slice(c * NC_PTS, (c + 1) * NC_PTS)
        h_full[0, :, sl, :] = hc
        y_full[0, :, sl, :] = yc
    return y_full, h_full


def run_on_hw(inputs, **spmd_kwargs):
    params = pack_params(inputs)
    xs = shard_x(inputs["x"])
    in_maps = [{"xin": xs[c], **params} for c in range(NCORES)]
    res = run_bass_kernel_spmd(_get_nc(), in_maps, list(range(NCORES)), **spmd_kwargs)
    return res


def kernel(**inputs):
    res = run_on_hw(inputs)
    return unshard(res.results)
